# revision 4
# baseline (speedup 1.0000x reference)
"""Trainium2 Bass kernel for nn_CCM: per-pixel complex 3x3 conv mask.

Math (per batch element b, sharded 1 batch element per NeuronCore):
  y[t,f] = sum_{c=0..26} m[c,t,f] * (w_{k(c)} * X)[t+i(c)-2, f+j(c)-1]
where c = 9*k + 3*i + j, w_k = v[0,k] + 1j*v[1,k] (cube roots of unity),
X = xr + 1j*xi, zero padded (causal in t: 2 top; symmetric in f: 1,1).

Layout: t = 8*p + tau, partitions p in [0,125), (tau, f) in the free dim,
so every tap shift is a free-dim offset read of padded "U" planes
U_k = w_k * X stored as [125, 10 tau-slots, 259 f-cols] (slots tau=-2..7).
"""

import sys
import numpy as np

sys.path.insert(0, "/opt/trn_rl_repo")

B = 8
C = 27
T = 1000
F = 257
TP = 125          # partitions
TAU = 8           # t = 8*p + tau
NS = 10           # tau slots in U planes: tau in [-2, 8)
FP = 259          # padded f width: f in [-1, 258)
SQ3H = float(np.sqrt(3.0) / 2.0)

_CACHE = {}


def _emit(ctx, tc, m_ap, x_ap, id_ap, y_ap):
    import concourse.mybir as mybir

    nc = tc.nc
    f32 = mybir.dt.float32
    FCS = [(0, 128), (128, 128), (256, 1)]  # f chunks for transposes

    const = ctx.enter_context(tc.tile_pool(name="const", bufs=1))
    planes = ctx.enter_context(tc.tile_pool(name="planes", bufs=1))
    mpool = ctx.enter_context(tc.tile_pool(name="mtiles", bufs=3))
    work = ctx.enter_context(tc.tile_pool(name="work", bufs=3))
    psum = ctx.enter_context(tc.tile_pool(name="psum", bufs=3, space="PSUM"))

    ident = const.tile([128, 128], f32, tag="ident")
    nc.sync.dma_start(ident[:], id_ap)

    # ---- load x in natural layout: [f, (tt, comp)] with tt = t + 2 (2 zero rows)
    xns = []
    for (f0, fw) in FCS:
        xn = const.tile([fw, (T + 2) * 2], f32, tag=f"xn{f0}")
        nc.vector.memset(xn[:, 0:4], 0.0)
        nc.sync.dma_start(
            xn[:, 4:], x_ap[f0:f0 + fw].rearrange("f t c -> f (t c)")
        )
        xns.append(xn)

    # ---- transpose x into blocked padded planes xr, xi: [TP, NS, FP]
    xq = []
    for q in range(2):
        p = planes.tile([TP, NS, FP], f32, tag=f"xq{q}")
        nc.vector.memset(p[:], 0.0)
        xq.append(p)
    for q in range(2):
        for ts in range(NS):  # slot ts corresponds to tau = ts - 2; tt = 8p + ts
            for ci, (f0, fw) in enumerate(FCS):
                pt = psum.tile([TP, 128], f32, tag="tp")
                xn3 = xns[ci].rearrange("f (t c) -> f t c", c=2)
                nc.tensor.transpose(
                    pt[0:TP, 0:fw],
                    xn3[0:fw, ts:ts + TAU * (TP - 1) + 1:TAU, q],
                    ident[0:fw, 0:fw],
                )
                nc.scalar.copy(xq[q][:, ts, 1 + f0:1 + f0 + fw], pt[0:TP, 0:fw])

    # ---- U planes: U_k = w_k * (xr + i xi), w_k = exp(+-2pi i/3), w_0 = 1
    mult = mybir.AluOpType.mult
    add = mybir.AluOpType.add
    sub = mybir.AluOpType.subtract
    t1 = planes.tile([TP, NS, FP], f32, tag="t1")
    t2 = planes.tile([TP, NS, FP], f32, tag="t2")
    ur1 = planes.tile([TP, NS, FP], f32, tag="ur1")
    ui1 = planes.tile([TP, NS, FP], f32, tag="ui1")
    ur2 = planes.tile([TP, NS, FP], f32, tag="ur2")
    ui2 = planes.tile([TP, NS, FP], f32, tag="ui2")
    nc.vector.tensor_scalar_mul(t1[:], xq[1][:], SQ3H)  # xi * s
    nc.vector.tensor_scalar_mul(t2[:], xq[0][:], SQ3H)  # xr * s
    nc.vector.scalar_tensor_tensor(ur1[:], xq[0][:], -0.5, t1[:], op0=mult, op1=sub)
    nc.vector.scalar_tensor_tensor(ui1[:], xq[1][:], -0.5, t2[:], op0=mult, op1=add)
    nc.vector.scalar_tensor_tensor(ur2[:], xq[0][:], -0.5, t1[:], op0=mult, op1=add)
    nc.vector.scalar_tensor_tensor(ui2[:], xq[1][:], -0.5, t2[:], op0=mult, op1=sub)
    U = [(xq[0], xq[1]), (ur1, ui1), (ur2, ui2)]

    # ---- tap loop: acc += m_c * U_k[shifted]
    acc_r = planes.tile([TP, TAU, F], f32, tag="accr")
    acc_i = planes.tile([TP, TAU, F], f32, tag="acci")
    for c in range(C):
        kk, n = divmod(c, 9)
        i, j = divmod(n, 3)
        dt, df = i - 2, j - 1
        mt = mpool.tile([TP, TAU * F], f32, tag="mt")
        nc.sync.dma_start(mt[:], m_ap[c].rearrange("(p t) f -> p (t f)", p=TP))
        m3 = mt.rearrange("p (t f) -> p t f", f=F)
        ur, ui = U[kk]
        urs = ur[:, dt + 2:dt + 2 + TAU, df + 1:df + 1 + F]
        uis = ui[:, dt + 2:dt + 2 + TAU, df + 1:df + 1 + F]
        if c == 0:
            nc.vector.tensor_mul(acc_r[:], m3[:], urs)
            nc.vector.tensor_mul(acc_i[:], m3[:], uis)
        else:
            pr = work.tile([TP, TAU, F], f32, tag="prod")
            nc.vector.tensor_mul(pr[:], m3[:], urs)
            nc.vector.tensor_add(acc_r[:], acc_r[:], pr[:])
            pi = work.tile([TP, TAU, F], f32, tag="prod")
            nc.vector.tensor_mul(pi[:], m3[:], uis)
            nc.vector.tensor_add(acc_i[:], acc_i[:], pi[:])

    # ---- transpose back to [f, (t, comp)] and store
    for ci, (f0, fw) in enumerate(FCS):
        yo = const.tile([fw, T * 2], f32, tag=f"yo{f0}")
        yv = yo.rearrange("f (t c) -> f t c", c=2)
        for comp, acc in ((0, acc_r), (1, acc_i)):
            for ts in range(TAU):
                pt = psum.tile([128, TP], f32, tag="tp2")
                nc.tensor.transpose(
                    pt[0:fw, 0:TP], acc[:, ts, f0:f0 + fw], ident[0:TP, 0:TP]
                )
                nc.scalar.copy(
                    yv[0:fw, ts:ts + TAU * (TP - 1) + 1:TAU, comp], pt[0:fw, 0:TP]
                )
        nc.sync.dma_start(y_ap[f0:f0 + fw].rearrange("f t c -> f (t c)"), yo[:])


def _build():
    if "nc" in _CACHE:
        return _CACHE["nc"]
    from contextlib import ExitStack
    from concourse import bacc, mybir
    import concourse.tile as tile

    f32 = mybir.dt.float32
    nc = bacc.Bacc("TRN2", target_bir_lowering=False, debug=False, num_devices=B)
    m_d = nc.dram_tensor("m", (C, T, F), f32, kind="ExternalInput")
    x_d = nc.dram_tensor("x", (F, T, 2), f32, kind="ExternalInput")
    id_d = nc.dram_tensor("ident", (128, 128), f32, kind="ExternalInput")
    y_d = nc.dram_tensor("y", (F, T, 2), f32, kind="ExternalOutput")

    with tile.TileContext(nc) as tc:
        with ExitStack() as ctx:
            _emit(ctx, tc, m_d.ap(), x_d.ap(), id_d.ap(), y_d.ap())
    nc.compile()
    _CACHE["nc"] = nc
    return nc


def _in_maps(m, x):
    ident = np.eye(128, dtype=np.float32)
    return [
        {"m": np.ascontiguousarray(m[b]), "x": np.ascontiguousarray(x[b]),
         "ident": ident}
        for b in range(B)
    ]


def kernel(m, x, v, _trace=False):
    from concourse import bass_utils

    m = np.asarray(m, dtype=np.float32)
    x = np.asarray(x, dtype=np.float32)
    nc = _build()
    res = bass_utils.run_bass_kernel_spmd(
        nc, _in_maps(m, x), core_ids=list(range(B)), trace=_trace
    )
    kernel.last_results = res
    y = np.stack([res.results[b]["y"] for b in range(B)], axis=0)
    return y
